# revision 70
# baseline (speedup 1.0000x reference)
"""Multi-head causal attention (B=4, T=2048, D=1024, H=16, hd=64) on 8 trn2 cores.

Sharding: core = (batch, head_group): 4 batches x 2 head-groups of 8 heads.
Each core computes its batch's attention for its 8 heads plus the partial
output projection; the host sums the two head-group partials per batch and
adds the output bias.

Projections (QKV and output) run in fp8-e4m3 with DoubleRow perf mode
(two interleaved contraction rows per PE pass); attention S/PV matmuls and
the softmax run in bf16. PSUM accumulation is fp32 throughout. The kernel
is a software pipeline: attention runs query-chunk-major (qc = 512
queries) over head pairs m, token-half-major overall, with the next pair's
projections (and later the output projection) interleaved into the
attention loop as closures so the tensor engine never idles while the
scalar engine computes the softmax exps (the per-iteration critical
resource). The two S-matmul halves of a head pair run concurrently on
disjoint PE row-halves. Softmax normalization is batched per (token-half,
pair) and dripped into the next attention loop in small stages; its only
gpsimd op is partition_broadcast so the DSP ucode library never swaps.

  xsb8 [128, k2, r, tok]     x^T fp8, contract f = k2*256 + r*128 + p
  qt/kt[128, m, tok]         bf16, partitions = half*64 + hd for pair m
  v_sb [128, tt, h, 65]      bf16 token-partition V, col 64 = ones
  es   [128, 2, 512]         bf16 exp(S^T), both heads of the pair
  ctx8 [128, ct2, r, tok]    fp8 context, feature = ct2*256 + r*128 + p
  ctx  [65, 512] PSUM        row 64 = softmax denominator via ones column
"""

import os
import sys

sys.path.insert(0, "/opt/trn_rl_repo")

import numpy as np

B = 4
T = 2048
D = 1024
H = 16
HD = 64
NCORES = 8
HPC = 8          # heads per core
DPC = HPC * HD   # 512
KT = D // 128    # 8 k-tiles
NT = T // 128    # 16 token tiles
M = 4            # head pairs per core

_CACHE = {}
LAST_RESULTS = None


def _build_program():
    from contextlib import ExitStack

    import concourse.bass as bass
    import concourse.tile as tile
    from concourse import bacc, mybir

    f32 = mybir.dt.float32
    bf16 = mybir.dt.bfloat16
    fp8 = mybir.dt.float8e4
    DR = mybir.MatmulPerfMode.DoubleRow
    Exp = mybir.ActivationFunctionType.Exp

    nc = bacc.Bacc(
        "TRN2", target_bir_lowering=False, debug=False, num_devices=NCORES
    )
    # host-prepacked layouts (see make_in_maps)
    x_d = nc.dram_tensor("x", [4 * 128, 4096], fp8, kind="ExternalInput").ap()
    wq_d = nc.dram_tensor("wq", [128, M * 1024], fp8, kind="ExternalInput").ap()
    wk_d = nc.dram_tensor("wk", [128, M * 1024], fp8, kind="ExternalInput").ap()
    wv_d = nc.dram_tensor("wv", [128, M * 1024], fp8, kind="ExternalInput").ap()
    wo_d = nc.dram_tensor("wo", [128, 4096], fp8, kind="ExternalInput").ap()
    out_d = nc.dram_tensor("out", [T, D], bf16, kind="ExternalOutput").ap()

    with tile.TileContext(nc) as tc, ExitStack() as top:
        persist = top.enter_context(tc.tile_pool(name="persist", bufs=1))
        xsb = persist.tile([128, 4, 2, T], fp8, tag="xsb")
        wqs = persist.tile([128, M, 4, 2, 128], fp8, tag="wqs")
        wks = persist.tile([128, M, 4, 2, 128], fp8, tag="wks")
        wvs = persist.tile([128, 4, 2, M, 128], fp8, tag="wvs")
        wos = persist.tile([128, 2, 2, D], fp8, tag="wos")
        qt = persist.tile([128, M, T], bf16, tag="qt")
        kt = persist.tile([128, M, T], bf16, tag="kt")
        v_sb = persist.tile([128, NT, HPC, HD + 1], bf16, tag="v")
        ctx8 = persist.tile([128, 2, 2, T], fp8, tag="ctx8")
        tri2 = persist.tile([128, 2, 128], bf16, tag="tri2")

        # ones columns feed the softmax-denominator row of the PV matmul
        nc.vector.memset(v_sb[:, :, :, HD : HD + 1], 1.0)
        # causal keep-mask for the 128-wide diagonal block, one copy per
        # half-plane: tri2[p, :, q] = 1 if q >= p else 0
        nc.vector.memset(tri2, 1.0)
        for i in range(2):
            nc.gpsimd.affine_select(
                out=tri2[:, i, :],
                in_=tri2[:, i, :],
                compare_op=mybir.AluOpType.is_ge,
                fill=0.0,
                base=0,
                pattern=[[1, 128]],
                channel_multiplier=-1,
            )

        # input DMAs round-robin across the DMA queues, ordered by when the
        # pipeline consumes each piece: the first QK chunk's k2-pieces of
        # wq/wk/x interleave so its contraction loop starts ASAP
        wq_r = wq_d.rearrange("p (m a r d) -> p m a r d", m=M, a=4, r=2)
        wk_r = wk_d.rearrange("p (m a r d) -> p m a r d", m=M, a=4, r=2)
        wv_r = wv_d.rearrange("p (a r m d) -> p a r m d", a=4, r=2, m=M)
        jobs = []
        for k2 in range(4):
            # four jobs per k2 over three queues — the rotation staggers
            # naturally, and the prologue's V projection gets its weight
            # slices alongside the first QK chunk's pieces
            jobs.append(
                (
                    xsb[:, k2, :, 0:512],
                    x_d[0:128, 1024 * k2 : 1024 * (k2 + 1)],
                )
            )
            jobs.append((wqs[:, 0, k2], wq_r[:, 0, k2]))
            jobs.append((wks[:, 0, k2], wk_r[:, 0, k2]))
            jobs.append((wvs[:, k2], wv_r[:, k2]))
        for ci in range(1, 4):
            for kh in range(2):
                jobs.append(
                    (
                        xsb[:, 2 * kh : 2 * kh + 2, :, 512 * ci : 512 * (ci + 1)],
                        x_d[
                            128 * ci : 128 * (ci + 1),
                            2048 * kh : 2048 * (kh + 1),
                        ],
                    )
                )
        for m in range(1, M):
            for w_sb, w_r in ((wqs, wq_r), (wks, wk_r)):
                jobs.append((w_sb[:, m], w_r[:, m]))
        jobs.append((wos, wo_d.rearrange("p (c r o) -> p c r o", c=2, r=2)))
        dqueues = [nc.sync, nc.scalar, nc.gpsimd]
        for i, job in enumerate(jobs):
            q = job[2] if len(job) > 2 else i % 3
            dqueues[q].dma_start(out=job[0], in_=job[1])

        pss = top.enter_context(tc.tile_pool(name="pss", bufs=3, space="PSUM"))
        psc = top.enter_context(tc.tile_pool(name="psc", bufs=2, space="PSUM"))
        expp = top.enter_context(tc.tile_pool(name="expp", bufs=6))
        smallp = top.enter_context(tc.tile_pool(name="smallp", bufs=3))
        outp = top.enter_context(tc.tile_pool(name="outp", bufs=3))

        def qk_chunk(m, ci):
            """Project Q^T and K^T for head pair m over tokens [512ci, 512ci+512)."""
            ps = pss.tile([128, 2, 512], f32, tag="pp", name=f"pqk_{m}_{ci}")
            for j, (w_sb, dest) in enumerate(((wqs, qt), (wks, kt))):
                for k2 in range(4):
                    nc.tensor.matmul(
                        ps[:, j, :],
                        w_sb[:, m, k2],
                        xsb[:, k2, :, 512 * ci : 512 * (ci + 1)],
                        start=(k2 == 0),
                        stop=(k2 == 3),
                        perf_mode=DR,
                    )
                nc.vector.tensor_copy(
                    dest[:, m, 512 * ci : 512 * (ci + 1)], ps[:, j, :]
                )

        def v_all(tt):
            """Project V for all head pairs over token tile tt — one wide
            moving operand per contraction step so the 256-row stationary
            loads hide under the matmuls."""
            ps = pss.tile([128, 2, 512], f32, tag="pp", name=f"pv_{tt}")
            for k2 in range(4):
                nc.tensor.matmul(
                    ps[:, 0, :],
                    xsb[:, k2, :, 128 * tt : 128 * (tt + 1)],
                    wvs[:, k2],
                    start=(k2 == 0),
                    stop=(k2 == 3),
                    perf_mode=DR,
                )
            nc.vector.tensor_copy(
                v_sb[:, tt, :, 0:HD],
                ps[:, 0, :].rearrange("p (h c) -> p h c", c=HD),
            )

        oqueues = [nc.sync, nc.gpsimd]

        def out_tile(tt):
            """Output projection for token tile tt (all four head pairs)."""
            ps = pss.tile([128, 2, 512], f32, tag="pp", name=f"po_{tt}")
            for oc in range(2):
                for ct2 in range(2):
                    nc.tensor.matmul(
                        ps[:, oc, :],
                        ctx8[:, ct2, :, 128 * tt : 128 * (tt + 1)],
                        wos[:, ct2, :, 512 * oc : 512 * (oc + 1)],
                        start=(ct2 == 0),
                        stop=(ct2 == 1),
                        perf_mode=DR,
                    )
            ot = outp.tile([128, 1024], bf16, tag="ot", name=f"ot_{tt}")
            otv = ot.rearrange("p (a b) -> p a b", a=2)
            if tt >= 12:
                # epilogue tiles: the scalar engine is idle by then, the
                # vector queue is busy with the final normalize multiplies
                nc.scalar.copy(otv, ps)
            else:
                nc.vector.tensor_copy(otv, ps)
            oqueues[tt % 2].dma_start(
                out=out_d[128 * tt : 128 * (tt + 1), :], in_=ot
            )

        def norm_stages(raw, m, c, qi=None):
            """Batched softmax normalization for (token half c, pair m):
            spread the denominators over 128 lanes via a small DMA reshape
            (a single-lane reciprocal would be ~8 cycles/elem), reshape the
            reciprocals back, broadcast them (the only steady-state gpsimd
            compute op, so its DSP ucode library stays resident), and scale
            the context into its fp8 layout on DVE. The first DMA is
            emitted inline; the rest is returned as drip stages. With qi
            set, process only that 512-query half."""
            tag = f"{m}_{c}" if qi is None else f"{m}_{c}_{qi}"
            qs = slice(0, 2) if qi is None else slice(qi, qi + 1)
            nq = 2 if qi is None else 1
            sp16 = smallp.tile([128, 16], f32, tag="sp16", name=f"sp_{tag}")
            if qi is None:
                nc.sync.dma_start(out=sp16, in_=raw[64:65, :, :, :])
            else:
                for h in range(2):
                    nc.sync.dma_start(
                        out=sp16[:, 4 * h : 4 * h + 4],
                        in_=raw[64:65, h, qi, :],
                    )
            rp16 = smallp.tile([128, 16], f32, tag="rp16", name=f"rp_{tag}")
            recip = smallp.tile([1, 2, 2, 512], f32, tag="recip", name=f"rc_{tag}")
            bcr = smallp.tile([64, 2, 2, 512], f32, tag="bcr", name=f"bc_{tag}")

            def s_recip():
                nc.vector.reciprocal(rp16[:, 0 : 8 * nq], sp16[:, 0 : 8 * nq])

            def s_row():
                if qi is None:
                    nc.sync.dma_start(out=recip, in_=rp16)
                else:
                    for h in range(2):
                        nc.sync.dma_start(
                            out=recip[:, h, qi, :],
                            in_=rp16[:, 4 * h : 4 * h + 4],
                        )

            def s_bcast():
                nc.gpsimd.partition_broadcast(
                    bcr[:, :, qs, :], recip[:, :, qs, :]
                )

            def s_mul(h):
                nc.vector.tensor_mul(
                    ctx8[
                        64 * h : 64 * h + 64,
                        m // 2,
                        m % 2,
                        1024 * c + 512 * (qi or 0) : 1024 * c
                        + 512 * (qi or 0)
                        + 512 * nq,
                    ].rearrange("p (a b) -> p a b", a=nq),
                    raw[0:64, h, qs, :],
                    bcr[:, h, qs, :],
                )

            return [s_recip, s_row, s_bcast, lambda: s_mul(0), lambda: s_mul(1)]

        pending = []  # deferred norm stages

        def attn(m, c, qi, raw, closures, pop_iters, it_base, late=None):
            """Causal attention for head pair m, queries [qc, qc+512)."""
            qc = 1024 * c + 512 * qi
            ctxp = [
                psc.tile([65, 512], f32, tag="ctx", name=f"ctx_{m}_{qc}_{i}")
                for i in range(2)
            ]
            tmax = qc // 128 + 3
            pv_prev = None

            def emit_pv(t, col0, w, es):
                for half in range(2):
                    nc.tensor.matmul(
                        ctxp[half][:, col0 - qc : col0 - qc + w],
                        v_sb[:, t, 2 * m + half, :],
                        es[:, half, 0:w],
                        start=(t == 0),
                        stop=(t == tmax),
                    )

            for t in range(tmax + 1):
                j0 = 128 * t
                col0 = max(j0, qc)
                w = qc + 512 - col0
                sp = pss.tile([128, 2, 512], f32, tag="pp", name=f"sp_{m}_{qc}_{t}")
                for half in range(2):
                    pr = 64 * half
                    nc.tensor.matmul(
                        sp[:, half, 0:w],
                        kt[pr : pr + 64, m, j0 : j0 + 128],
                        qt[pr : pr + 64, m, col0 : col0 + w],
                        start=True,
                        stop=True,
                    )
                es = expp.tile(
                    [128, 2, 512], bf16, tag="es", name=f"es_{m}_{qc}_{t}"
                )
                nc.scalar.activation(
                    es[:, :, 0:w], sp[:, :, 0:w], Exp, scale=0.125
                )
                if col0 == j0:
                    # zero the upper triangle of the diagonal block
                    nc.vector.tensor_mul(es[:, :, 0:128], es[:, :, 0:128], tri2)
                # emit the PREVIOUS iteration's PV here: its exp has had a
                # full iteration to complete, so it never head-of-line
                # blocks the PE queue while this iteration's S runs
                if pv_prev is not None:
                    emit_pv(*pv_prev)
                pv_prev = (t, col0, w, es)
                # drip deferred normalize stages, one per iteration
                if pending and t >= 1:
                    pending.pop(0)()
                if (it_base + t) in pop_iters and closures:
                    closures.pop(0)()
                if late and t >= tmax - 3:
                    late.pop(0)()
            emit_pv(*pv_prev)
            # free the PSUM accumulators promptly (gates the psc ring)
            for h in range(2):
                nc.vector.tensor_copy(raw[:, h, qi, :], ctxp[h])

        # prologue: only what the very first attention iterations need —
        # the rest of the projections ride in as closures so the scalar
        # engine starts on softmax exps as early as possible
        qk_chunk(0, 0)
        v_all(0)
        v_all(1)

        c0_closures = [
            [
                (lambda: v_all(2)),
                (lambda: qk_chunk(0, 1)),
                (lambda: v_all(3)),
                (lambda: v_all(4)),
                (lambda: v_all(5)),
                (lambda: qk_chunk(0, 2)),
                (lambda: v_all(6)),
                (lambda: v_all(7)),
                (lambda: qk_chunk(0, 3)),
            ]
            + [(lambda cc=ci: qk_chunk(1, cc)) for ci in range(3)],
            [(lambda: qk_chunk(1, 3)), (lambda: v_all(8)), (lambda: v_all(9))]
            + [(lambda cc=ci: qk_chunk(2, cc)) for ci in range(4)],
            [(lambda: v_all(10)), (lambda: v_all(11))]
            + [(lambda cc=ci: qk_chunk(3, cc)) for ci in range(4)],
            [(lambda t=j: v_all(t)) for j in range(12, 16)],
        ]

        # main pipeline: token-half-major, head pair m inner, 512-query
        # chunks innermost. Closures keep the PE fed during the ACT-paced
        # attention iterations; all V projections land in c=0 so the c=1
        # vector queue carries no PV-critical copies.
        for c in range(2):
            for m in range(M):
                if c == 0:
                    closures = c0_closures[m]
                else:
                    # first-half output tiles, ready once (c0, m3) has
                    # normalized (its stages pop early in (c1, m0))
                    base, nout = [(0, 2), (2, 3), (5, 2), (7, 1)][m]
                    closures = [
                        (lambda t=base + j: out_tile(t)) for j in range(nout)
                    ]
                n_iters = 12 if c == 0 else 28
                n_cl = len(closures)
                if c == 1 and m == 0:
                    pop_iters = {14, 21}
                else:
                    pop_iters = {i * n_iters // n_cl for i in range(n_cl)}
                raw = smallp.tile(
                    [65, 2, 2, 512], f32, tag="raw", bufs=3, name=f"raw_{m}_{c}"
                )
                last = c == 1 and m == M - 1
                it_base = 0
                for qi in range(2):
                    # in the very last chunk, emit the output tiles that
                    # depend only on the already-normalized first query half
                    late = (
                        [(lambda t=j: out_tile(t)) for j in range(8, 12)]
                        if last and qi == 1
                        else None
                    )
                    attn(m, c, qi, raw, closures, pop_iters, it_base, late)
                    it_base += (1024 * c + 512 * qi) // 128 + 4
                    if last:
                        # per-half chains so the first overlaps the second
                        # half's attention, shortening the final drain
                        for stage in norm_stages(raw, m, c, qi=qi):
                            stage()
                if last:
                    # keep the PE clocked up through the final normalize
                    # chain so the epilogue matmuls run at full p-state
                    for j in range(7):
                        wp = pss.tile(
                            [128, 2, 512], f32, tag="pp", name=f"warm_{j}"
                        )
                        nc.tensor.matmul(
                            wp[:, 0, :],
                            raw[0:64, 0, 0, 0:128],
                            raw[0:64, :, :, :].rearrange("p a b c -> p (a b c)")[
                                :, 0:512
                            ],
                            start=True,
                            stop=True,
                        )
                while closures:
                    closures.pop(0)()
                if not last:
                    pending.extend(norm_stages(raw, m, c))
        while pending:
            pending.pop(0)()

        # epilogue: remaining output tiles
        for tt in range(12, NT):
            out_tile(tt)

    nc.compile()
    return nc


def _get_program():
    if "nc" not in _CACHE:
        _CACHE["nc"] = _build_program()
    return _CACHE["nc"]


def make_in_maps(x, Wq, Wk, Wv, Wo):
    import ml_dtypes

    fp8 = ml_dtypes.float8_e4m3
    in_maps = []
    for core in range(NCORES):
        b, hg = core // 2, core % 2
        sl = slice(DPC * hg, DPC * (hg + 1))
        # x: [ci*128+p, k2*1024 + r*512 + tok] = x[b].T[k2*256+r*128+p, 512ci+tok]
        xb = np.ascontiguousarray(x[b].T).astype(fp8)
        xr = (
            xb.reshape(4, 2, 128, 4, 512)
            .transpose(3, 2, 0, 1, 4)
            .reshape(512, 4096)
        )
        # wq/wk: [p, m*1024 + k2*256 + r*128 + d] = W[k2*256+r*128+p, m*128+d]
        def pack_w(W):
            return np.ascontiguousarray(
                W[:, sl]
                .reshape(4, 2, 128, M, 128)
                .transpose(2, 3, 0, 1, 4)
                .reshape(128, M * 1024)
            ).astype(fp8)

        # wv: k2-major so one moving operand covers all pairs:
        # [p, k2*1024 + r*512 + m*128 + d]
        wv = np.ascontiguousarray(
            Wv[:, sl]
            .reshape(4, 2, 128, M, 128)
            .transpose(2, 0, 1, 3, 4)
            .reshape(128, M * 1024)
        ).astype(fp8)

        # wo: [p, ct2*2048 + r*1024 + o] = Wo[sl][ct2*256+r*128+p, o]
        wo = np.ascontiguousarray(
            Wo[sl, :]
            .reshape(2, 2, 128, D)
            .transpose(2, 0, 1, 3)
            .reshape(128, 4096)
        ).astype(fp8)
        in_maps.append(
            {
                "x": np.ascontiguousarray(xr),
                "wq": pack_w(Wq),
                "wk": pack_w(Wk),
                "wv": wv,
                "wo": wo,
            }
        )
    return in_maps


def kernel(x, Wq, Wk, Wv, Wo, bo):
    global LAST_RESULTS
    from concourse.bass_utils import run_bass_kernel_spmd

    x = np.asarray(x, dtype=np.float32)
    nc = _get_program()
    in_maps = make_in_maps(
        x,
        np.asarray(Wq, np.float32),
        np.asarray(Wk, np.float32),
        np.asarray(Wv, np.float32),
        np.asarray(Wo, np.float32),
    )
    res = run_bass_kernel_spmd(
        nc,
        in_maps,
        list(range(NCORES)),
        trace=bool(int(os.environ.get("KERNEL_TRACE", "0"))),
    )
    LAST_RESULTS = res
    bo = np.asarray(bo, np.float32)
    out = np.empty((B, T, D), np.float32)
    for b in range(B):
        out[b] = (
            res.results[2 * b]["out"].astype(np.float32)
            + res.results[2 * b + 1]["out"].astype(np.float32)
            + bo
        )
    return out


# revision 76
# speedup vs baseline: 1.0223x; 1.0223x over previous
"""Multi-head causal attention (B=4, T=2048, D=1024, H=16, hd=64) on 8 trn2 cores.

Sharding: core = (batch, head_group): 4 batches x 2 head-groups of 8 heads.
Each core computes its batch's attention for its 8 heads plus the partial
output projection; the host sums the two head-group partials per batch and
adds the output bias.

Projections (QKV and output) run in fp8-e4m3 with DoubleRow perf mode
(two interleaved contraction rows per PE pass); attention S/PV matmuls and
the softmax run in bf16. PSUM accumulation is fp32 throughout. The kernel
is a software pipeline: attention runs query-chunk-major (qc = 512
queries) over head pairs m, token-half-major overall, with the next pair's
projections (and later the output projection) interleaved into the
attention loop as closures so the tensor engine never idles while the
scalar engine computes the softmax exps (the per-iteration critical
resource). The two S-matmul halves of a head pair run concurrently on
disjoint PE row-halves. Softmax normalization is batched per (token-half,
pair) and dripped into the next attention loop in small stages; its only
gpsimd op is partition_broadcast so the DSP ucode library never swaps.

  xsb8 [128, k2, r, tok]     x^T fp8, contract f = k2*256 + r*128 + p
  qt/kt[128, m, tok]         bf16, partitions = half*64 + hd for pair m
  v_sb [128, tt, h, 65]      bf16 token-partition V, col 64 = ones
  es   [128, 2, 512]         bf16 exp(S^T), both heads of the pair
  ctx8 [128, ct2, r, tok]    fp8 context, feature = ct2*256 + r*128 + p
  ctx  [65, 512] PSUM        row 64 = softmax denominator via ones column
"""

import os
import sys

sys.path.insert(0, "/opt/trn_rl_repo")

import numpy as np

B = 4
T = 2048
D = 1024
H = 16
HD = 64
NCORES = 8
HPC = 8          # heads per core
DPC = HPC * HD   # 512
KT = D // 128    # 8 k-tiles
NT = T // 128    # 16 token tiles
M = 4            # head pairs per core

_CACHE = {}
LAST_RESULTS = None


def _build_program():
    from contextlib import ExitStack

    import concourse.bass as bass
    import concourse.tile as tile
    from concourse import bacc, mybir

    f32 = mybir.dt.float32
    bf16 = mybir.dt.bfloat16
    fp8 = mybir.dt.float8e4
    DR = mybir.MatmulPerfMode.DoubleRow
    Exp = mybir.ActivationFunctionType.Exp

    nc = bacc.Bacc(
        "TRN2", target_bir_lowering=False, debug=False, num_devices=NCORES
    )
    # host-prepacked layouts (see make_in_maps)
    x_d = nc.dram_tensor("x", [4 * 128, 4096], fp8, kind="ExternalInput").ap()
    wq_d = nc.dram_tensor("wq", [128, M * 1024], fp8, kind="ExternalInput").ap()
    wk_d = nc.dram_tensor("wk", [128, M * 1024], fp8, kind="ExternalInput").ap()
    wv_d = nc.dram_tensor("wv", [128, M * 1024], fp8, kind="ExternalInput").ap()
    wo_d = nc.dram_tensor("wo", [128, 4096], fp8, kind="ExternalInput").ap()
    out_d = nc.dram_tensor("out", [T, D], bf16, kind="ExternalOutput").ap()

    with tile.TileContext(nc) as tc, ExitStack() as top:
        persist = top.enter_context(tc.tile_pool(name="persist", bufs=1))
        xsb = persist.tile([128, 4, 2, T], fp8, tag="xsb")
        wqs = persist.tile([128, M, 4, 2, 128], fp8, tag="wqs")
        wks = persist.tile([128, M, 4, 2, 128], fp8, tag="wks")
        wvs = persist.tile([128, 4, 2, M, 128], fp8, tag="wvs")
        wos = persist.tile([128, 2, 2, D], fp8, tag="wos")
        qt = persist.tile([128, M, T], bf16, tag="qt")
        kt = persist.tile([128, M, T], bf16, tag="kt")
        v_sb = persist.tile([128, NT, HPC, HD + 1], bf16, tag="v")
        ctx8 = persist.tile([128, 2, 2, T], fp8, tag="ctx8")
        tri2 = persist.tile([128, 2, 128], bf16, tag="tri2")

        # ones columns feed the softmax-denominator row of the PV matmul
        nc.vector.memset(v_sb[:, :, :, HD : HD + 1], 1.0)
        # causal keep-mask for the 128-wide diagonal block, one copy per
        # half-plane: tri2[p, :, q] = 1 if q >= p else 0
        nc.vector.memset(tri2, 1.0)
        for i in range(2):
            nc.gpsimd.affine_select(
                out=tri2[:, i, :],
                in_=tri2[:, i, :],
                compare_op=mybir.AluOpType.is_ge,
                fill=0.0,
                base=0,
                pattern=[[1, 128]],
                channel_multiplier=-1,
            )

        # input DMAs round-robin across the DMA queues, ordered by when the
        # pipeline consumes each piece: the first QK chunk's k2-pieces of
        # wq/wk/x interleave so its contraction loop starts ASAP
        wq_r = wq_d.rearrange("p (m a r d) -> p m a r d", m=M, a=4, r=2)
        wk_r = wk_d.rearrange("p (m a r d) -> p m a r d", m=M, a=4, r=2)
        wv_r = wv_d.rearrange("p (a r m d) -> p a r m d", a=4, r=2, m=M)
        jobs = []
        for k2 in range(4):
            # four jobs per k2 over three queues — the rotation staggers
            # naturally, and the prologue's V projection gets its weight
            # slices alongside the first QK chunk's pieces
            jobs.append(
                (
                    xsb[:, k2, :, 0:512],
                    x_d[0:128, 1024 * k2 : 1024 * (k2 + 1)],
                )
            )
            jobs.append((wqs[:, 0, k2], wq_r[:, 0, k2]))
            jobs.append((wks[:, 0, k2], wk_r[:, 0, k2]))
            jobs.append((wvs[:, k2], wv_r[:, k2]))
        for ci in range(1, 4):
            for kh in range(2):
                jobs.append(
                    (
                        xsb[:, 2 * kh : 2 * kh + 2, :, 512 * ci : 512 * (ci + 1)],
                        x_d[
                            128 * ci : 128 * (ci + 1),
                            2048 * kh : 2048 * (kh + 1),
                        ],
                    )
                )
        for m in range(1, M):
            for w_sb, w_r in ((wqs, wq_r), (wks, wk_r)):
                jobs.append((w_sb[:, m], w_r[:, m]))
        jobs.append((wos, wo_d.rearrange("p (c r o) -> p c r o", c=2, r=2)))
        dqueues = [nc.sync, nc.scalar, nc.gpsimd]
        for i, job in enumerate(jobs):
            q = job[2] if len(job) > 2 else i % 3
            dqueues[q].dma_start(out=job[0], in_=job[1])

        pss = top.enter_context(tc.tile_pool(name="pss", bufs=3, space="PSUM"))
        psc = top.enter_context(tc.tile_pool(name="psc", bufs=2, space="PSUM"))
        expp = top.enter_context(tc.tile_pool(name="expp", bufs=6))
        smallp = top.enter_context(tc.tile_pool(name="smallp", bufs=3))
        outp = top.enter_context(tc.tile_pool(name="outp", bufs=3))

        def qk_chunk(m, ci):
            """Project Q^T and K^T for head pair m over tokens [512ci, 512ci+512)."""
            ps = pss.tile([128, 2, 512], f32, tag="pp", name=f"pqk_{m}_{ci}")
            for j, (w_sb, dest) in enumerate(((wqs, qt), (wks, kt))):
                for k2 in range(4):
                    nc.tensor.matmul(
                        ps[:, j, :],
                        w_sb[:, m, k2],
                        xsb[:, k2, :, 512 * ci : 512 * (ci + 1)],
                        start=(k2 == 0),
                        stop=(k2 == 3),
                        perf_mode=DR,
                    )
                nc.vector.tensor_copy(
                    dest[:, m, 512 * ci : 512 * (ci + 1)], ps[:, j, :]
                )

        def v_all(tt):
            """Project V for all head pairs over token tile tt — one wide
            moving operand per contraction step so the 256-row stationary
            loads hide under the matmuls."""
            ps = pss.tile([128, 2, 512], f32, tag="pp", name=f"pv_{tt}")
            for k2 in range(4):
                nc.tensor.matmul(
                    ps[:, 0, :],
                    xsb[:, k2, :, 128 * tt : 128 * (tt + 1)],
                    wvs[:, k2],
                    start=(k2 == 0),
                    stop=(k2 == 3),
                    perf_mode=DR,
                )
            nc.vector.tensor_copy(
                v_sb[:, tt, :, 0:HD],
                ps[:, 0, :].rearrange("p (h c) -> p h c", c=HD),
            )

        oqueues = [nc.sync, nc.gpsimd]

        def out_tile(tt):
            """Output projection for token tile tt (all four head pairs)."""
            ps = pss.tile([128, 2, 512], f32, tag="pp", name=f"po_{tt}")
            for oc in range(2):
                for ct2 in range(2):
                    nc.tensor.matmul(
                        ps[:, oc, :],
                        ctx8[:, ct2, :, 128 * tt : 128 * (tt + 1)],
                        wos[:, ct2, :, 512 * oc : 512 * (oc + 1)],
                        start=(ct2 == 0),
                        stop=(ct2 == 1),
                        perf_mode=DR,
                    )
            ot = outp.tile([128, 1024], bf16, tag="ot", name=f"ot_{tt}")
            otv = ot.rearrange("p (a b) -> p a b", a=2)
            if tt >= 12:
                # epilogue tiles: the scalar engine is idle by then, the
                # vector queue is busy with the final normalize multiplies
                nc.scalar.copy(otv, ps)
            else:
                nc.vector.tensor_copy(otv, ps)
            oqueues[tt % 2].dma_start(
                out=out_d[128 * tt : 128 * (tt + 1), :], in_=ot
            )

        def norm_stages(raw, m, c, qi=None):
            """Batched softmax normalization for (token half c, pair m):
            spread the denominators over 128 lanes via a small DMA reshape
            (a single-lane reciprocal would be ~8 cycles/elem), reshape the
            reciprocals back, broadcast them (the only steady-state gpsimd
            compute op, so its DSP ucode library stays resident), and scale
            the context into its fp8 layout on DVE. The first DMA is
            emitted inline; the rest is returned as drip stages. With qi
            set, process only that 512-query half."""
            tag = f"{m}_{c}" if qi is None else f"{m}_{c}_{qi}"
            qs = slice(0, 2) if qi is None else slice(qi, qi + 1)
            nq = 2 if qi is None else 1
            sp16 = smallp.tile([128, 16], f32, tag="sp16", name=f"sp_{tag}")

            def s_sp():
                if qi is None:
                    nc.sync.dma_start(out=sp16, in_=raw[64:65, :, :, :])
                else:
                    for h in range(2):
                        nc.sync.dma_start(
                            out=sp16[:, 4 * h : 4 * h + 4],
                            in_=raw[64:65, h, qi, :],
                        )

            rp16 = smallp.tile([128, 16], f32, tag="rp16", name=f"rp_{tag}")
            recip = smallp.tile([1, 2, 2, 512], f32, tag="recip", name=f"rc_{tag}")
            bcr = smallp.tile([64, 2, 2, 512], f32, tag="bcr", name=f"bc_{tag}")

            def s_recip():
                nc.vector.reciprocal(rp16[:, 0 : 8 * nq], sp16[:, 0 : 8 * nq])

            def s_row():
                if qi is None:
                    nc.sync.dma_start(out=recip, in_=rp16)
                else:
                    for h in range(2):
                        nc.sync.dma_start(
                            out=recip[:, h, qi, :],
                            in_=rp16[:, 4 * h : 4 * h + 4],
                        )

            def s_bcast():
                nc.gpsimd.partition_broadcast(
                    bcr[:, :, qs, :], recip[:, :, qs, :]
                )

            def s_mul(h):
                nc.vector.tensor_mul(
                    ctx8[
                        64 * h : 64 * h + 64,
                        m // 2,
                        m % 2,
                        1024 * c + 512 * (qi or 0) : 1024 * c
                        + 512 * (qi or 0)
                        + 512 * nq,
                    ].rearrange("p (a b) -> p a b", a=nq),
                    raw[0:64, h, qs, :],
                    bcr[:, h, qs, :],
                )

            return [s_sp, s_recip, s_row, s_bcast, lambda: s_mul(0), lambda: s_mul(1)]

        pending = []  # deferred norm stages
        carry = []  # previous chunk's final PV + PSUM-freeing copies

        def attn(m, c, qi, raw, closures, pop_iters, it_base, late=None, defer=True):
            """Causal attention for head pair m, queries [qc, qc+512)."""
            qc = 1024 * c + 512 * qi
            ctxp = [
                psc.tile([65, 512], f32, tag="ctx", name=f"ctx_{m}_{qc}_{i}")
                for i in range(2)
            ]
            tmax = qc // 128 + 3
            pv_prev = None

            def emit_pv(t, col0, w, es):
                for half in range(2):
                    nc.tensor.matmul(
                        ctxp[half][:, col0 - qc : col0 - qc + w],
                        v_sb[:, t, 2 * m + half, :],
                        es[:, half, 0:w],
                        start=(t == 0),
                        stop=(t == tmax),
                    )

            for t in range(tmax + 1):
                j0 = 128 * t
                col0 = max(j0, qc)
                w = qc + 512 - col0
                sp = pss.tile([128, 2, 512], f32, tag="pp", name=f"sp_{m}_{qc}_{t}")
                for half in range(2):
                    pr = 64 * half
                    nc.tensor.matmul(
                        sp[:, half, 0:w],
                        kt[pr : pr + 64, m, j0 : j0 + 128],
                        qt[pr : pr + 64, m, col0 : col0 + w],
                        start=True,
                        stop=True,
                    )
                es = expp.tile(
                    [128, 2, 512], bf16, tag="es", name=f"es_{m}_{qc}_{t}"
                )
                nc.scalar.activation(
                    es[:, :, 0:w], sp[:, :, 0:w], Exp, scale=0.125
                )
                if col0 == j0:
                    # zero the upper triangle of the diagonal block
                    nc.vector.tensor_mul(es[:, :, 0:128], es[:, :, 0:128], tri2)
                # emit the PREVIOUS iteration's PV here: its exp has had a
                # full iteration to complete, so it never head-of-line
                # blocks the PE queue while this iteration's S runs. The
                # first iteration instead flushes the previous CHUNK's
                # deferred tail for the same reason.
                if pv_prev is not None:
                    emit_pv(*pv_prev)
                elif carry:
                    carry.pop(0)()
                pv_prev = (t, col0, w, es)
                # drip deferred normalize stages, one per iteration
                if pending and t >= 1:
                    pending.pop(0)()
                if (it_base + t) in pop_iters and closures:
                    closures.pop(0)()
                if late and t >= tmax - 3:
                    late.pop(0)()
            def tail(pv=pv_prev, cp=ctxp):
                emit_pv(*pv)
                # free the PSUM accumulators (gates the psc ring)
                for h in range(2):
                    nc.vector.tensor_copy(raw[:, h, qi, :], cp[h])

            if defer:
                carry.append(tail)
            else:
                tail()

        # prologue: only what the very first attention iterations need —
        # the rest of the projections ride in as closures so the scalar
        # engine starts on softmax exps as early as possible
        qk_chunk(0, 0)
        v_all(0)
        v_all(1)

        c0_closures = [
            [
                (lambda: v_all(2)),
                (lambda: qk_chunk(0, 1)),
                (lambda: v_all(3)),
                (lambda: v_all(4)),
                (lambda: v_all(5)),
                (lambda: qk_chunk(0, 2)),
                (lambda: v_all(6)),
                (lambda: v_all(7)),
                (lambda: qk_chunk(0, 3)),
            ]
            + [(lambda cc=ci: qk_chunk(1, cc)) for ci in range(3)],
            [(lambda: qk_chunk(1, 3)), (lambda: v_all(8)), (lambda: v_all(9))]
            + [(lambda cc=ci: qk_chunk(2, cc)) for ci in range(4)],
            [(lambda: v_all(10)), (lambda: v_all(11))]
            + [(lambda cc=ci: qk_chunk(3, cc)) for ci in range(4)],
            [(lambda t=j: v_all(t)) for j in range(12, 16)],
        ]

        # main pipeline: token-half-major, head pair m inner, 512-query
        # chunks innermost. Closures keep the PE fed during the ACT-paced
        # attention iterations; all V projections land in c=0 so the c=1
        # vector queue carries no PV-critical copies.
        for c in range(2):
            for m in range(M):
                if c == 0:
                    closures = c0_closures[m]
                else:
                    # first-half output tiles, ready once (c0, m3) has
                    # normalized (its stages pop early in (c1, m0))
                    base, nout = [(0, 2), (2, 3), (5, 2), (7, 1)][m]
                    closures = [
                        (lambda t=base + j: out_tile(t)) for j in range(nout)
                    ]
                n_iters = 12 if c == 0 else 28
                n_cl = len(closures)
                if c == 1 and m == 0:
                    pop_iters = {14, 21}
                else:
                    pop_iters = {i * n_iters // n_cl for i in range(n_cl)}
                raw = smallp.tile(
                    [65, 2, 2, 512], f32, tag="raw", bufs=3, name=f"raw_{m}_{c}"
                )
                last = c == 1 and m == M - 1
                it_base = 0
                for qi in range(2):
                    # in the very last chunk, emit the output tiles that
                    # depend only on the already-normalized first query half
                    late = (
                        [(lambda t=j: out_tile(t)) for j in range(8, 12)]
                        if last and qi == 1
                        else None
                    )
                    attn(
                        m, c, qi, raw, closures, pop_iters, it_base, late,
                        defer=not last,
                    )
                    it_base += (1024 * c + 512 * qi) // 128 + 4
                    if last:
                        # per-half chains so the first overlaps the second
                        # half's attention, shortening the final drain
                        for stage in norm_stages(raw, m, c, qi=qi):
                            stage()
                if last:
                    # keep the PE clocked up through the final normalize
                    # chain so the epilogue matmuls run at full p-state
                    for j in range(7):
                        wp = pss.tile(
                            [128, 2, 512], f32, tag="pp", name=f"warm_{j}"
                        )
                        nc.tensor.matmul(
                            wp[:, 0, :],
                            raw[0:64, 0, 0, 0:128],
                            raw[0:64, :, :, :].rearrange("p a b c -> p (a b c)")[
                                :, 0:512
                            ],
                            start=True,
                            stop=True,
                        )
                while closures:
                    closures.pop(0)()
                if not last:
                    pending.extend(norm_stages(raw, m, c))
        while pending:
            pending.pop(0)()

        # epilogue: remaining output tiles
        for tt in range(12, NT):
            out_tile(tt)

    nc.compile()
    return nc


def _get_program():
    if "nc" not in _CACHE:
        _CACHE["nc"] = _build_program()
    return _CACHE["nc"]


def make_in_maps(x, Wq, Wk, Wv, Wo):
    import ml_dtypes

    fp8 = ml_dtypes.float8_e4m3
    in_maps = []
    for core in range(NCORES):
        b, hg = core // 2, core % 2
        sl = slice(DPC * hg, DPC * (hg + 1))
        # x: [ci*128+p, k2*1024 + r*512 + tok] = x[b].T[k2*256+r*128+p, 512ci+tok]
        xb = np.ascontiguousarray(x[b].T).astype(fp8)
        xr = (
            xb.reshape(4, 2, 128, 4, 512)
            .transpose(3, 2, 0, 1, 4)
            .reshape(512, 4096)
        )
        # wq/wk: [p, m*1024 + k2*256 + r*128 + d] = W[k2*256+r*128+p, m*128+d]
        def pack_w(W):
            return np.ascontiguousarray(
                W[:, sl]
                .reshape(4, 2, 128, M, 128)
                .transpose(2, 3, 0, 1, 4)
                .reshape(128, M * 1024)
            ).astype(fp8)

        # wv: k2-major so one moving operand covers all pairs:
        # [p, k2*1024 + r*512 + m*128 + d]
        wv = np.ascontiguousarray(
            Wv[:, sl]
            .reshape(4, 2, 128, M, 128)
            .transpose(2, 0, 1, 3, 4)
            .reshape(128, M * 1024)
        ).astype(fp8)

        # wo: [p, ct2*2048 + r*1024 + o] = Wo[sl][ct2*256+r*128+p, o]
        wo = np.ascontiguousarray(
            Wo[sl, :]
            .reshape(2, 2, 128, D)
            .transpose(2, 0, 1, 3)
            .reshape(128, 4096)
        ).astype(fp8)
        in_maps.append(
            {
                "x": np.ascontiguousarray(xr),
                "wq": pack_w(Wq),
                "wk": pack_w(Wk),
                "wv": wv,
                "wo": wo,
            }
        )
    return in_maps


def kernel(x, Wq, Wk, Wv, Wo, bo):
    global LAST_RESULTS
    from concourse.bass_utils import run_bass_kernel_spmd

    x = np.asarray(x, dtype=np.float32)
    nc = _get_program()
    in_maps = make_in_maps(
        x,
        np.asarray(Wq, np.float32),
        np.asarray(Wk, np.float32),
        np.asarray(Wv, np.float32),
        np.asarray(Wo, np.float32),
    )
    res = run_bass_kernel_spmd(
        nc,
        in_maps,
        list(range(NCORES)),
        trace=bool(int(os.environ.get("KERNEL_TRACE", "0"))),
    )
    LAST_RESULTS = res
    bo = np.asarray(bo, np.float32)
    out = np.empty((B, T, D), np.float32)
    for b in range(B):
        out[b] = (
            res.results[2 * b]["out"].astype(np.float32)
            + res.results[2 * b + 1]["out"].astype(np.float32)
            + bo
        )
    return out
